# revision 14
# baseline (speedup 1.0000x reference)
"""FFM cell kernel for Trainium2, 8 NeuronCores, batch-parallel.

Math: per batch element b,
    gated[t,m] = (x@W_pre + b_pre)[t,m] * sigmoid(x@W_gin + b_gin)[t,m]
    state[t,m,c] = sum_{s<=t} exp((a_m + i*b_c)*(t-s)) * gated[s,m]
    zm = [state.re, state.im] @ W_mix + b_mix
    out = LN(zm * sig(gout)) + skip * (1 - sig(gout))

The complex diagonal recurrence is decoupled into two *real* first-order
scans using angle addition (z is real):
    A[t,ch] = e^{a_m} A[t-1,ch] + cos(b_c t) z[t,m]
    B[t,ch] = e^{a_m} B[t-1,ch] + sin(b_c t) z[t,m]
    state_re = cos(b_c t) A + sin(b_c t) B
    state_im = sin(b_c t) A - cos(b_c t) B
mapped onto the DVE hardware scan (tensor_tensor_scan), channels (m,c) on
partitions, time on the free dim; cos/sin tables host-precomputed.
Matmuls run as float32r (1 cycle/row vs 4 for float32). The elementwise
stream around the scans optionally runs in bf16 (DVE 2x mode).
Sharding: batch element -> core; everything replicated; no collectives.
"""

import numpy as np

B, T, D = 8, 1024, 512
TR, CTX, OUT = 64, 16, 512
EPS = 1e-6
NCH = TR * CTX   # 1024 scan channels
NG = NCH // 128  # 8 channel groups of 128 partitions
NT = T // 128    # 8 token tiles
KD = D // 128    # 4 contraction chunks over D

STREAM_BF16 = True   # bf16 modulation/post stream (DVE 2x) vs fp32
STAGES = "ALL"       # ablation: "A" (loads+gates), "B" (+scans), "ALL"
POOL_SCAN = False    # GPSIMD scan rejected by ISA check (DVE-only op)
# TAIL_V2: LN tail via STT-with-accum (h product + free-dim sum in one
# DVE op), variance from ACT Square+accum, bf16 tail tensors.  Measured
# 143.9us vs 157.3us median pair slope (2001x10) -- keep True.
TAIL_V2 = True
# POOL_IM/POOL_P2: moving p4/sim (or p2) products to GPSIMD measured
# 173.7us (vs 143.9 with TAIL_V2 alone): Pool tensor ops in kernel
# context cost ~2.7us+ each and serialize the im-chain.  Keep False.
POOL_IM = False
POOL_P2 = False

_cache = {}


def build_program(n_rep=1, with_bias=True, loop_n=1):
    """Build + compile the Bass program (single NEFF, SPMD on 8 cores).

    n_rep > 1 repeats the whole pipeline (incl. DMA loads) for
    differential wall-clock timing; each repeat rewrites the output.
    loop_n > 1 wraps the body in a tc.For_i hardware loop instead
    (no instruction replication) for high-amplification timing."""
    import concourse.bacc as bacc
    import concourse.tile as tile
    import concourse.mybir as mybir
    from concourse.alu_op_type import AluOpType as op

    f32 = mybir.dt.float32
    f32r = mybir.dt.float32r
    bf16 = mybir.dt.bfloat16
    sdt = bf16 if STREAM_BF16 else f32
    AF = mybir.ActivationFunctionType

    def r(ap):  # fp32 -> fp32r view for fast PE matmul
        return ap.bitcast(f32r) if ap.dtype == f32 else ap

    wb = with_bias
    nc = bacc.Bacc("TRN2", target_bir_lowering=False, debug=False)

    def din(name, shape, dt=f32):
        return nc.dram_tensor(name, shape, dt, kind="ExternalInput").ap()

    zx_dma = globals().get("ZX_DMA", True)
    gs_fp8 = globals().get("GS_FP8", True)
    f8 = mybir.dt.float8e4
    PM = mybir.MatmulPerfMode
    DSC = 2.0 ** -16                      # 1/(SX*SW) descale for fp8 paths
    xT = din("xT", (D, T), sdt)
    Wpg = din("Wpg", (D, 2 * TR), sdt)    # [W_pre | W_gin] packed
    if gs_fp8:
        # fp8 DoubleRow operands, layout [Ki=128, Ko=KD, *] (d = Ko*128+Ki)
        xT8 = din("xT8", (128, KD, T), f8)
        Wg8 = din("Wg8", (128, KD, OUT), f8)
        Ws8 = din("Ws8", (128, KD, OUT), f8)
    if not zx_dma:
        EXPM = din("EXPM", (TR, NCH), sdt)
    Wgout = din("Wgout", (D, OUT), sdt)
    Wskip = din("Wskip", (D, OUT), sdt)
    Wmre = din("Wmre", (NCH, OUT), sdt)   # W_mix real rows, (m,c) order
    Wmim = din("Wmim", (NCH, OUT), sdt)
    COS = din("COS", (128, T), sdt)       # row rr: cos(b_{rr%16} * t)
    SIN = din("SIN", (128, T), sdt)
    DEC = din("DEC", (128, NG))           # col g: exp(-|a_{8g + rr//16}|)
    # materialized decay operand: scan with a stride-1 bf16 data0 is
    # ~0.5us/op faster than the free-dim-broadcast AP (slope-measured)
    DECF = din("DECF", (128, NG * T), sdt)
    bpre = din("bpre", (TR, 1))
    bgin = din("bgin", (TR, 1))
    bgout = din("bgout", (1, OUT), f32r)
    bskip = din("bskip", (1, OUT), f32r)
    bmix = din("bmix", (1, OUT), f32r)
    ones = din("ones", (1, 128), f32r)
    out_d = nc.dram_tensor("out", (T, OUT), f32, kind="ExternalOutput").ap()

    from contextlib import ExitStack

    with tile.TileContext(nc) as tc:
     with ExitStack() as _loop_ctx:
      if loop_n > 1:
          _loop_ctx.enter_context(tc.For_i(0, loop_n, 1))
      for _rep in range(n_rep):
        with (
            tc.tile_pool(name="singles", bufs=1) as singles,
            tc.tile_pool(name="states", bufs=1) as states,
        ):
            def load(ap_dram, shape, tag, dt=f32, q=nc.sync):
                t = singles.tile(shape, dt, tag=tag, name=tag)
                q.dma_start(out=t, in_=ap_dram)
                return t

            xT_sb = [load(xT[k * 128:(k + 1) * 128, :], [128, T], f"xT{k}", sdt)
                     for k in range(KD)]
            Wpg_sb = [load(Wpg[k * 128:(k + 1) * 128, :], [128, 2 * TR],
                           f"wpg{k}", sdt) for k in range(KD)]
            if gs_fp8:
                xT8_sb = load(xT8, [128, KD, T], "xT8", f8)
                Wg8_sb = load(Wg8, [128, KD, OUT], "wg8", f8, nc.scalar)
                Ws8_sb = load(Ws8, [128, KD, OUT], "ws8", f8, nc.scalar)
            else:
                Wgout_sb = [load(Wgout[k * 128:(k + 1) * 128, :], [128, OUT],
                                 f"wgout{k}", sdt, nc.scalar)
                            for k in range(KD)]
                Wskip_sb = [load(Wskip[k * 128:(k + 1) * 128, :], [128, OUT],
                                 f"wskip{k}", sdt, nc.scalar)
                            for k in range(KD)]
            Wmre_sb = [load(Wmre[g * 128:(g + 1) * 128, :], [128, OUT],
                            f"wmre{g}", sdt, nc.scalar) for g in range(NG)]
            Wmim_sb = [load(Wmim[g * 128:(g + 1) * 128, :], [128, OUT],
                            f"wmim{g}", sdt, nc.scalar) for g in range(NG)]
            COS_sb = load(COS, [128, T], "cos", sdt)
            SIN_sb = load(SIN, [128, T], "sin", sdt)
            DEC_sb = load(DEC, [128, NG], "dec")
            DECF_sb = singles.tile([128, NG * T], sdt, tag="decf",
                                   name="decf")
            for g in range(NG):
                nc.scalar.dma_start(out=DECF_sb[:, g * T:(g + 1) * T],
                                    in_=DECF[:, g * T:(g + 1) * T])
            if not zx_dma:
                EXPM_sb = load(EXPM, [TR, NCH], "expm", sdt)
            bpre_sb = load(bpre, [TR, 1], "bpre")
            bgin_sb = load(bgin, [TR, 1], "bgin")
            bgout_sb = load(bgout, [1, OUT], "bgout", f32r)
            bskip_sb = load(bskip, [1, OUT], "bskip", f32r)
            bmix_sb = load(bmix, [1, OUT], "bmix", f32r)

            ones_sb = load(ones, [1, 128], "ones", f32r)
            eps_sb = singles.tile([128, 1], f32, tag="eps")
            nc.vector.memset(eps_sb, EPS)

            sre = [states.tile([128, T], sdt, tag=f"sre{g}", name=f"sre{g}")
                   for g in range(NG)]
            sim = [states.tile([128, T], sdt, tag=f"sim{g}", name=f"sim{g}")
                   for g in range(NG)]
            gdt = sdt if TAIL_V2 else f32
            gsigs = [states.tile([128, OUT], gdt, tag=f"gsig{ti}",
                                 name=f"gsig{ti}") for ti in range(NT)]
            skips = [states.tile([128, OUT], gdt, tag=f"skip{ti}",
                                 name=f"skip{ti}") for ti in range(NT)]

            # ---- stage A: gated = (pre + bpre) * sig(gin + bgin) ----
            # [W_pre | W_gin] packed on partitions: one matmul fills both
            gated = singles.tile([TR, T], sdt, tag="gated")
            with (
                tc.tile_pool(name="psumA", bufs=1, space="PSUM") as psumA,
                tc.tile_pool(name="wkA", bufs=2) as wkA,
            ):
                pg_ps = psumA.tile([2 * TR, T], f32, tag="pg")
                for h in range(2):
                    cols = slice(h * 512, (h + 1) * 512)
                    for k in range(KD):
                        nc.tensor.matmul(pg_ps[:, cols], Wpg_sb[k],
                                         xT_sb[k][:, cols],
                                         start=(k == 0), stop=(k == KD - 1))
                gsigA = wkA.tile([TR, T], f32, tag="gsigA")
                for h in range(2):
                    cols = slice(h * 512, (h + 1) * 512)
                    nc.scalar.activation(gsigA[:, cols],
                                         pg_ps[TR:2 * TR, cols],
                                         AF.Sigmoid, bias=bgin_sb, scale=1.0)
                nc.vector.scalar_tensor_tensor(
                    out=gated, in0=pg_ps[0:TR, :], scalar=bpre_sb, in1=gsigA,
                    op0=op.add, op1=op.mult)

            NTE = globals().get("NTE_OVERRIDE", 7)  # stage-B psum tiles
            # ---- stage B: scans per channel group + gout/skip fill ----
            if STAGES == "A":
                fin = states.tile([128, OUT], f32, tag="fin", name="fin")
                nc.vector.memset(fin, 0.5)
                nc.vector.scalar_tensor_tensor(
                    out=fin, in0=pre_ps if False else fin, scalar=bpre_sb[0:1, 0:1] if False else 1.0,
                    in1=fin, op0=op.mult, op1=op.mult)
                nc.sync.dma_start(out=out_d[0:128, :], in_=fin)
            if STAGES != "A":
              with (
                tc.tile_pool(name="psumG", bufs=1, space="PSUM") as psumG,
                tc.tile_pool(name="psumM", bufs=1, space="PSUM") as psumM,
                tc.tile_pool(name="wkC", bufs=3) as wkC,
                tc.tile_pool(name="wkB", bufs=3) as wkB,
            ):
                zms = [psumM.tile([128, OUT], f32, tag=f"zm{ti}",
                                  name=f"zm{ti}") for ti in range(NTE)]

                def ln_tail(ti, zm, wk=None):
                    wk = wk if wk is not None else wkC
                    gsig = gsigs[ti]
                    if TAIL_V2:
                        # h via STT with free-dim sum; var from ACT
                        # Square+accum; bf16 tail ops (DVE 2x)
                        h_t = wk.tile([128, OUT], sdt, tag="h", name="h_t")
                        sumh = wk.tile([128, 1], f32, tag="sh",
                                       name="sumh")
                        nc.vector.scalar_tensor_tensor(
                            out=h_t, in0=zm, scalar=1.0, in1=gsig,
                            op0=op.mult, op1=op.mult, accum_out=sumh)
                        sq = wk.tile([128, OUT], sdt, tag="sqs", name="sq")
                        sumh2 = wk.tile([128, 1], f32, tag="sh2",
                                        name="sumh2")
                        nc.scalar.activation(sq, h_t, AF.Square,
                                             accum_out=sumh2)
                        mu = wk.tile([128, 1], f32, tag="mu", name="mu")
                        nc.vector.tensor_scalar(mu, sumh, 1.0 / OUT, None,
                                                op.mult)
                        q0 = wk.tile([128, 1], f32, tag="q0", name="q0")
                        nc.vector.tensor_tensor(q0, mu, sumh, op.mult)
                        q1 = wk.tile([128, 1], f32, tag="q1", name="q1")
                        nc.vector.tensor_tensor(q1, q0, sumh2, op.subtract)
                        sd = wk.tile([128, 1], f32, tag="sd", name="sd")
                        nc.scalar.activation(sd, q1, AF.Sqrt, bias=eps_sb,
                                             scale=-1.0 / OUT)
                        rstd = wk.tile([128, 1], f32, tag="rstd",
                                       name="rstd")
                        nc.vector.reciprocal(rstd, sd)
                        beta = wk.tile([128, 1], f32, tag="beta",
                                       name="beta")
                        nc.vector.scalar_tensor_tensor(
                            out=beta, in0=mu, scalar=-1.0, in1=rstd,
                            op0=op.mult, op1=op.mult)
                        ln = wk.tile([128, OUT], sdt, tag="ln", name="ln")
                        nc.scalar.activation(ln, h_t, AF.Identity,
                                             bias=beta, scale=rstd)
                        omg = wk.tile([128, OUT], sdt, tag="omg",
                                      name="omg")
                        nc.scalar.activation(omg, gsig, AF.Copy,
                                             bias=1.0, scale=-1.0)
                        sk2 = wk.tile([128, OUT], sdt, tag="sk2",
                                      name="sk2")
                        nc.vector.tensor_tensor(sk2, omg, skips[ti],
                                                op.mult)
                        outt = wk.tile([128, OUT], f32, tag="outt",
                                       name="outt")
                        nc.vector.tensor_tensor(outt, ln, sk2, op.add)
                        nc.sync.dma_start(
                            out=out_d[ti * 128:(ti + 1) * 128, :],
                            in_=outt)
                        return
                    h_t = wk.tile([128, OUT], f32, tag="h", name="h_t")
                    nc.vector.tensor_tensor(h_t, gsig, zm, op.mult)
                    stats = wk.tile([128, 6], f32, tag="stats", name="stats")
                    nc.vector.bn_stats(stats, h_t)
                    mv = wk.tile([128, 2], f32, tag="mv", name="mv")
                    nc.vector.bn_aggr(mv, stats)
                    sd = wk.tile([128, 1], f32, tag="sd", name="sd")
                    nc.scalar.activation(sd, mv[:, 1:2], AF.Sqrt,
                                         bias=eps_sb, scale=1.0)
                    rstd = wk.tile([128, 1], f32, tag="rstd", name="rstd")
                    nc.vector.reciprocal(rstd, sd)
                    beta = wk.tile([128, 1], f32, tag="beta", name="beta")
                    nc.vector.scalar_tensor_tensor(
                        out=beta, in0=mv[:, 0:1], scalar=-1.0, in1=rstd,
                        op0=op.mult, op1=op.mult)
                    ln = wk.tile([128, OUT], f32, tag="ln", name="ln")
                    nc.scalar.activation(ln, h_t, AF.Identity,
                                         bias=beta, scale=rstd)
                    omg = wk.tile([128, OUT], f32, tag="omg", name="omg")
                    nc.scalar.activation(omg, gsig, AF.Copy,
                                         bias=1.0, scale=-1.0)
                    sk2 = wk.tile([128, OUT], f32, tag="sk2", name="sk2")
                    nc.vector.tensor_tensor(sk2, omg, skips[ti], op.mult)
                    outt = wk.tile([128, OUT], f32, tag="outt", name="outt")
                    nc.vector.tensor_tensor(outt, ln, sk2, op.add)
                    nc.sync.dma_start(out=out_d[ti * 128:(ti + 1) * 128, :],
                                      in_=outt)
                for g in range(NG):
                    # broadcast gated rows m -> 16 c-rows via DMA (no PE)
                    zxs = wkB.tile([128, T], sdt, tag="zxs")
                    if zx_dma:
                        # SWDGE (gpsimd) queue: the HWDGE path splits one
                        # DMA across 16 SDMA engines and its completion
                        # semaphore can fire before all replicated writes
                        # land; the software-descgen path is safe.
                        nc.gpsimd.dma_start(
                            out=zxs,
                            in_=gated[8 * g:8 * g + 8, :].unsqueeze(1)
                            .broadcast_to((8, 16, T)))
                    else:
                        with tc.tile_pool(name="psumB", bufs=2,
                                          space="PSUM") as psumB:
                            for h in range(2):
                                cols = slice(h * 512, (h + 1) * 512)
                                zx = psumB.tile([128, 512], f32, tag="zx",
                                                name="zx")
                                nc.tensor.matmul(
                                    zx,
                                    EXPM_sb[:, g * 128:(g + 1) * 128],
                                    gated[:, cols], start=True, stop=True)
                                nc.scalar.activation(zxs[:, cols], zx,
                                                     AF.Copy)
                    inA = wkB.tile([128, T], sdt, tag="mod")
                    inB = wkB.tile([128, T], sdt, tag="mod")
                    nc.vector.tensor_tensor(inA, COS_sb, zxs, op.mult)
                    nc.vector.tensor_tensor(inB, SIN_sb, zxs, op.mult)
                    a_t = wkB.tile([128, T], sdt, tag="scn")
                    b_t = wkB.tile([128, T], sdt, tag="scn")
                    dec_b = DECF_sb[:, g * T:(g + 1) * T]
                    nc.vector.tensor_tensor_scan(
                        a_t, dec_b, inA, 0.0, op.mult, op.add)
                    (nc.gpsimd if POOL_SCAN else nc.vector).tensor_tensor_scan(
                        b_t, dec_b, inB, 0.0, op.mult, op.add)
                    # state_re = COS*A + SIN*B   (DVE)
                    p1 = wkB.tile([128, T], sdt, tag="mod")
                    p2 = wkB.tile([128, T], sdt, tag="mod")
                    nc.vector.tensor_tensor(p1, COS_sb, a_t, op.mult)
                    (nc.gpsimd if POOL_P2 else nc.vector).tensor_tensor(
                        p2, SIN_sb, b_t, op.mult)
                    nc.vector.tensor_tensor(sre[g], p1, p2, op.add)
                    # state_im = SIN*A - COS*B
                    PIM = nc.gpsimd if POOL_IM else nc.vector
                    p3 = wkB.tile([128, T], sdt, tag="pim")
                    p4 = wkB.tile([128, T], sdt, tag="pim")
                    nc.vector.tensor_tensor(p3, SIN_sb, a_t, op.mult)
                    PIM.tensor_tensor(p4, COS_sb, b_t, op.mult)
                    PIM.tensor_tensor(sim[g], p3, p4, op.subtract)
                    # gout/skip matmuls for token tile g fill PE idle time
                    ti = g
                    tcols = slice(ti * 128, (ti + 1) * 128)
                    gout_ps = psumG.tile([128, OUT], f32, tag="gout",
                                         name="gout_ps")
                    if gs_fp8:
                        for k in range(0, KD, 2):
                            nc.tensor.matmul(gout_ps,
                                             xT8_sb[:, k:k + 2, tcols],
                                             Wg8_sb[:, k:k + 2, :],
                                             start=(k == 0),
                                             stop=(not wb and k == KD - 2),
                                             perf_mode=PM.DoubleRow)
                    else:
                        for k in range(KD):
                            nc.tensor.matmul(gout_ps, xT_sb[k][:, tcols],
                                             Wgout_sb[k], start=(k == 0),
                                             stop=(not wb and k == KD - 1))
                    if wb:
                        nc.tensor.matmul(gout_ps, r(ones_sb), r(bgout_sb),
                                         start=False, stop=True)
                    nc.scalar.activation(gsigs[ti], gout_ps, AF.Sigmoid,
                                         scale=DSC if gs_fp8 else 1.0)
                    skip_ps = psumG.tile([128, OUT], f32, tag="gout",
                                         name="skip_ps")
                    if gs_fp8:
                        for k in range(0, KD, 2):
                            nc.tensor.matmul(skip_ps,
                                             xT8_sb[:, k:k + 2, tcols],
                                             Ws8_sb[:, k:k + 2, :],
                                             start=(k == 0),
                                             stop=(not wb and k == KD - 2),
                                             perf_mode=PM.DoubleRow)
                    else:
                        for k in range(KD):
                            nc.tensor.matmul(skip_ps, xT_sb[k][:, tcols],
                                             Wskip_sb[k], start=(k == 0),
                                             stop=(not wb and k == KD - 1))
                    if wb:
                        nc.tensor.matmul(skip_ps, r(ones_sb), r(bskip_sb),
                                         start=False, stop=True)
                    nc.scalar.activation(skips[ti], skip_ps, AF.Copy,
                                         scale=DSC if gs_fp8 else 1.0)
                    for tj in range(NTE):
                        tc2 = slice(tj * 128, (tj + 1) * 128)
                        nc.tensor.matmul(zms[tj], sre[g][:, tc2],
                                         Wmre_sb[g], start=(g == 0),
                                         stop=False, skip_group_check=True)
                        nc.tensor.matmul(zms[tj], sim[g][:, tc2],
                                         Wmim_sb[g], start=False,
                                         stop=(not wb and g == NG - 1),
                                         skip_group_check=True)
                    if g == NG - 1:
                        for tj in range(NTE):
                            if wb:
                                nc.tensor.matmul(zms[tj], r(ones_sb),
                                                 r(bmix_sb), start=False,
                                                 stop=True,
                                                 skip_group_check=True)
                            ln_tail(tj, zms[tj])

              if STAGES == "B":
                fin = states.tile([128, OUT], f32, tag="fin", name="fin")
                nc.vector.tensor_tensor(fin, skips[0], gsigs[0], op.mult)
                for g in range(NG):
                    nc.vector.tensor_tensor(fin, sre[g][:, 0:OUT],
                                            sim[g][:, 0:OUT], op.mult)
                nc.sync.dma_start(out=out_d[0:128, :], in_=fin)
            # ---- stage C: remaining mix tiles + LN tail ----
            if STAGES == "ALL":
              with (
                tc.tile_pool(name="psumC", bufs=3, space="PSUM") as psumC,
                tc.tile_pool(name="wkC2", bufs=3) as wkC2,
            ):
                for ti in range(NTE, NT):
                    tcols = slice(ti * 128, (ti + 1) * 128)
                    zm = psumC.tile([128, OUT], f32, tag="zm", name="zm")
                    for g in range(NG):
                        nc.tensor.matmul(zm, sre[g][:, tcols], Wmre_sb[g],
                                         start=(g == 0), stop=False)
                        nc.tensor.matmul(zm, sim[g][:, tcols], Wmim_sb[g],
                                         start=False,
                                         stop=(not wb and g == NG - 1))
                    if wb:
                        nc.tensor.matmul(zm, r(ones_sb), r(bmix_sb),
                                         start=False, stop=True)
                    ln_tail(ti, zm, wkC2)

    nc.compile()
    return nc


def host_prep(inputs):
    """Compute per-core input maps from the full problem inputs."""
    import ml_dtypes

    sdt_np = ml_dtypes.bfloat16 if STREAM_BF16 else np.float32

    x = np.asarray(inputs["x"], np.float32)
    a = np.abs(np.asarray(inputs["ffa_a"], np.float64))       # [TR]
    b = np.asarray(inputs["ffa_b"], np.float64)               # [CTX]
    t = np.arange(T, dtype=np.float64)

    cos_cols = np.cos(b[:, None] * t[None, :])                # [CTX, T]
    sin_cols = np.sin(b[:, None] * t[None, :])
    COS = np.tile(cos_cols, (8, 1)).astype(sdt_np)            # [128, T]
    SIN = np.tile(sin_cols, (8, 1)).astype(sdt_np)

    dec = np.exp(-a).astype(np.float32)                       # [TR]
    rr = np.arange(128)
    DEC = np.empty((128, NG), np.float32)
    for g in range(NG):
        DEC[:, g] = dec[8 * g + rr // 16]
    DECF = np.empty((128, NG * T), np.float32)
    for g in range(NG):
        DECF[:, g * T:(g + 1) * T] = DEC[:, g][:, None]

    Wm = np.asarray(inputs["W_mix"], np.float32).reshape(TR, 2, CTX, OUT)
    Wmre = np.ascontiguousarray(Wm[:, 0].reshape(NCH, OUT)).astype(sdt_np)
    Wmim = np.ascontiguousarray(Wm[:, 1].reshape(NCH, OUT)).astype(sdt_np)

    Wpg = np.concatenate(
        [np.asarray(inputs["W_pre"], np.float32),
         np.asarray(inputs["W_gin"], np.float32)], axis=1)

    col = np.arange(NCH)
    EXPM = (np.arange(TR)[:, None] == (col[None, :] // CTX)).astype(sdt_np)

    # fp8 DoubleRow operands: scale by powers of 2, descale 2^-16 on-chip
    SX, SW = 32.0, 2048.0
    f8 = ml_dtypes.float8_e4m3fn

    def dr_pack(a, scale):  # [D, N] -> [Ki=128, Ko=KD, N] with d = Ko*128+Ki
        q = (np.asarray(a, np.float32) * scale).astype(f8)
        return np.ascontiguousarray(
            q.reshape(KD, 128, a.shape[1]).transpose(1, 0, 2))

    Wg8 = dr_pack(inputs["W_gout"], SW)
    Ws8 = dr_pack(inputs["W_skip"], SW)

    shared = {
        "Wpg": np.ascontiguousarray(Wpg).astype(sdt_np),
        "EXPM": EXPM,
        "Wg8": Wg8, "Ws8": Ws8,
        "Wgout": np.ascontiguousarray(inputs["W_gout"], np.float32).astype(sdt_np),
        "Wskip": np.ascontiguousarray(inputs["W_skip"], np.float32).astype(sdt_np),
        "Wmre": Wmre, "Wmim": Wmim,
        "COS": COS, "SIN": SIN, "DEC": DEC,
        "DECF": DECF.astype(sdt_np),
        "bpre": np.asarray(inputs["b_pre"], np.float32).reshape(TR, 1),
        "bgin": np.asarray(inputs["b_gin"], np.float32).reshape(TR, 1),
        # fp8 path: PSUM holds (SX*SW)*x@W; Act applies 2^-16, so biases
        # folded in via the ones-matmul must be pre-scaled by 2^16.
        "bgout": np.asarray(inputs["b_gout"], np.float32).reshape(1, OUT)
        * (SX * SW),
        "bskip": np.asarray(inputs["b_skip"], np.float32).reshape(1, OUT)
        * (SX * SW),
        "bmix": np.asarray(inputs["b_mix"], np.float32).reshape(1, OUT),
        "ones": np.ones((1, 128), np.float32),
    }
    in_maps = []
    for core in range(B):
        m = dict(shared)
        xTc = np.ascontiguousarray(x[core].T)
        m["xT"] = xTc.astype(sdt_np)
        m["xT8"] = dr_pack(xTc, SX)
        in_maps.append(m)
    return in_maps


def kernel(**inputs):
    from concourse import bass_utils

    wb = any(
        np.any(np.asarray(inputs[k]))
        for k in ("b_pre", "b_gin", "b_gout", "b_skip", "b_mix")
    )
    key = f"nc_wb{wb}"
    if key not in _cache:
        _cache[key] = build_program(with_bias=wb)
    nc = _cache[key]
    in_maps = host_prep(inputs)
    res = bass_utils.run_bass_kernel_spmd(nc, in_maps, core_ids=list(range(B)))
    return np.stack([res.results[i]["out"] for i in range(B)])



# revision 20
# speedup vs baseline: 1.0929x; 1.0929x over previous
"""FFM cell kernel for Trainium2, 8 NeuronCores, batch-parallel.

Math: per batch element b,
    gated[t,m] = (x@W_pre + b_pre)[t,m] * sigmoid(x@W_gin + b_gin)[t,m]
    state[t,m,c] = sum_{s<=t} exp((a_m + i*b_c)*(t-s)) * gated[s,m]
    zm = [state.re, state.im] @ W_mix + b_mix
    out = LN(zm * sig(gout)) + skip * (1 - sig(gout))

The complex diagonal recurrence is decoupled into two *real* first-order
scans using angle addition (z is real):
    A[t,ch] = e^{a_m} A[t-1,ch] + cos(b_c t) z[t,m]
    B[t,ch] = e^{a_m} B[t-1,ch] + sin(b_c t) z[t,m]
    state_re = cos(b_c t) A + sin(b_c t) B
    state_im = sin(b_c t) A - cos(b_c t) B
mapped onto the DVE hardware scan (tensor_tensor_scan), channels (m,c) on
partitions, time on the free dim; cos/sin tables host-precomputed.
Matmuls run as float32r (1 cycle/row vs 4 for float32). The elementwise
stream around the scans optionally runs in bf16 (DVE 2x mode).
Sharding: batch element -> core; everything replicated; no collectives.
"""

import numpy as np

B, T, D = 8, 1024, 512
TR, CTX, OUT = 64, 16, 512
EPS = 1e-6
NCH = TR * CTX   # 1024 scan channels
NG = NCH // 128  # 8 channel groups of 128 partitions
NT = T // 128    # 8 token tiles
KD = D // 128    # 4 contraction chunks over D

STREAM_BF16 = True   # bf16 modulation/post stream (DVE 2x) vs fp32
STAGES = "ALL"       # ablation: "A" (loads+gates), "B" (+scans), "ALL"
POOL_SCAN = False    # GPSIMD scan rejected by ISA check (DVE-only op)
# TAIL_V2: LN tail via STT-with-accum (h product + free-dim sum in one
# DVE op), variance from ACT Square+accum, bf16 tail tensors.  Measured
# 143.9us vs 157.3us median pair slope (2001x10) -- keep True.
TAIL_V2 = True
# POOL_IM/POOL_P2: moving p4/sim (or p2) products to GPSIMD measured
# 173.7us (vs 143.9 with TAIL_V2 alone): Pool tensor ops in kernel
# context cost ~2.7us+ each and serialize the im-chain.  Keep False.
POOL_IM = False
POOL_P2 = False
# MOD_FUSE: post-modulation ops fused over group pairs ([128,2048] TT
# = 1293ns vs 2x810ns microbenched, -8us DVE predicted).  Slope-
# measured 133.9us median-pair (noisy window) vs 132.6-143.9 without:
# no regression signal, bit-identical output, same-engine change only.
MOD_FUSE = True

_cache = {}


def build_program(n_rep=1, with_bias=True, loop_n=1):
    """Build + compile the Bass program (single NEFF, SPMD on 8 cores).

    n_rep > 1 repeats the whole pipeline (incl. DMA loads) for
    differential wall-clock timing; each repeat rewrites the output.
    loop_n > 1 wraps the body in a tc.For_i hardware loop instead
    (no instruction replication) for high-amplification timing."""
    import concourse.bacc as bacc
    import concourse.tile as tile
    import concourse.mybir as mybir
    from concourse.alu_op_type import AluOpType as op

    f32 = mybir.dt.float32
    f32r = mybir.dt.float32r
    bf16 = mybir.dt.bfloat16
    sdt = bf16 if STREAM_BF16 else f32
    AF = mybir.ActivationFunctionType

    def r(ap):  # fp32 -> fp32r view for fast PE matmul
        return ap.bitcast(f32r) if ap.dtype == f32 else ap

    wb = with_bias
    nc = bacc.Bacc("TRN2", target_bir_lowering=False, debug=False)

    def din(name, shape, dt=f32):
        return nc.dram_tensor(name, shape, dt, kind="ExternalInput").ap()

    zx_dma = globals().get("ZX_DMA", True)
    gs_fp8 = globals().get("GS_FP8", True)
    f8 = mybir.dt.float8e4
    PM = mybir.MatmulPerfMode
    DSC = 2.0 ** -16                      # 1/(SX*SW) descale for fp8 paths
    xT = din("xT", (D, T), sdt)
    Wpg = din("Wpg", (D, 2 * TR), sdt)    # [W_pre | W_gin] packed
    if gs_fp8:
        # fp8 DoubleRow operands, layout [Ki=128, Ko=KD, *] (d = Ko*128+Ki)
        xT8 = din("xT8", (128, KD, T), f8)
        Wg8 = din("Wg8", (128, KD, OUT), f8)
        Ws8 = din("Ws8", (128, KD, OUT), f8)
    if not zx_dma:
        EXPM = din("EXPM", (TR, NCH), sdt)
    Wgout = din("Wgout", (D, OUT), sdt)
    Wskip = din("Wskip", (D, OUT), sdt)
    Wmre = din("Wmre", (NCH, OUT), sdt)   # W_mix real rows, (m,c) order
    Wmim = din("Wmim", (NCH, OUT), sdt)
    COS = din("COS", (128, T), sdt)       # row rr: cos(b_{rr%16} * t)
    SIN = din("SIN", (128, T), sdt)
    DEC = din("DEC", (128, NG))           # col g: exp(-|a_{8g + rr//16}|)
    # materialized decay operand: scan with a stride-1 bf16 data0 is
    # ~0.5us/op faster than the free-dim-broadcast AP (slope-measured)
    DECF = din("DECF", (128, NG * T), sdt)
    bpre = din("bpre", (TR, 1))
    bgin = din("bgin", (TR, 1))
    bgout = din("bgout", (1, OUT), f32r)
    bskip = din("bskip", (1, OUT), f32r)
    bmix = din("bmix", (1, OUT), f32r)
    ones = din("ones", (1, 128), f32r)
    out_d = nc.dram_tensor("out", (T, OUT), f32, kind="ExternalOutput").ap()

    from contextlib import ExitStack

    with tile.TileContext(nc) as tc:
     with ExitStack() as _loop_ctx:
      if loop_n > 1:
          _loop_ctx.enter_context(tc.For_i(0, loop_n, 1))
      for _rep in range(n_rep):
        with (
            tc.tile_pool(name="singles", bufs=1) as singles,
            tc.tile_pool(name="states", bufs=1) as states,
        ):
            def load(ap_dram, shape, tag, dt=f32, q=nc.sync):
                t = singles.tile(shape, dt, tag=tag, name=tag)
                q.dma_start(out=t, in_=ap_dram)
                return t

            xT_sb = [load(xT[k * 128:(k + 1) * 128, :], [128, T], f"xT{k}", sdt)
                     for k in range(KD)]
            Wpg_sb = [load(Wpg[k * 128:(k + 1) * 128, :], [128, 2 * TR],
                           f"wpg{k}", sdt) for k in range(KD)]
            if gs_fp8:
                xT8_sb = load(xT8, [128, KD, T], "xT8", f8)
                Wg8_sb = load(Wg8, [128, KD, OUT], "wg8", f8, nc.scalar)
                Ws8_sb = load(Ws8, [128, KD, OUT], "ws8", f8, nc.scalar)
            else:
                Wgout_sb = [load(Wgout[k * 128:(k + 1) * 128, :], [128, OUT],
                                 f"wgout{k}", sdt, nc.scalar)
                            for k in range(KD)]
                Wskip_sb = [load(Wskip[k * 128:(k + 1) * 128, :], [128, OUT],
                                 f"wskip{k}", sdt, nc.scalar)
                            for k in range(KD)]
            Wmre_sb = [load(Wmre[g * 128:(g + 1) * 128, :], [128, OUT],
                            f"wmre{g}", sdt, nc.scalar) for g in range(NG)]
            Wmim_sb = [load(Wmim[g * 128:(g + 1) * 128, :], [128, OUT],
                            f"wmim{g}", sdt, nc.scalar) for g in range(NG)]
            COS_sb = load(COS, [128, T], "cos", sdt)
            SIN_sb = load(SIN, [128, T], "sin", sdt)
            DEC_sb = load(DEC, [128, NG], "dec")
            DECF_sb = singles.tile([128, NG * T], sdt, tag="decf",
                                   name="decf")
            for g in range(NG):
                nc.scalar.dma_start(out=DECF_sb[:, g * T:(g + 1) * T],
                                    in_=DECF[:, g * T:(g + 1) * T])
            if not zx_dma:
                EXPM_sb = load(EXPM, [TR, NCH], "expm", sdt)
            bpre_sb = load(bpre, [TR, 1], "bpre")
            bgin_sb = load(bgin, [TR, 1], "bgin")
            bgout_sb = load(bgout, [1, OUT], "bgout", f32r)
            bskip_sb = load(bskip, [1, OUT], "bskip", f32r)
            bmix_sb = load(bmix, [1, OUT], "bmix", f32r)

            ones_sb = load(ones, [1, 128], "ones", f32r)
            eps_sb = singles.tile([128, 1], f32, tag="eps")
            nc.vector.memset(eps_sb, EPS)

            if MOD_FUSE:
                # pair tiles: group 2p in cols [0,T), 2p+1 in [T,2T)
                srep = [states.tile([128, 2 * T], sdt, tag=f"srep{p}",
                                    name=f"srep{p}") for p in range(NG // 2)]
                simp = [states.tile([128, 2 * T], sdt, tag=f"simp{p}",
                                    name=f"simp{p}") for p in range(NG // 2)]
                COS2_sb = singles.tile([128, 2 * T], sdt, tag="cos2",
                                       name="cos2")
                SIN2_sb = singles.tile([128, 2 * T], sdt, tag="sin2",
                                       name="sin2")
                for h in range(2):
                    nc.scalar.dma_start(
                        out=COS2_sb[:, h * T:(h + 1) * T], in_=COS)
                    nc.scalar.dma_start(
                        out=SIN2_sb[:, h * T:(h + 1) * T], in_=SIN)

                def sre_v(g, cols):
                    p, j = divmod(g, 2)
                    return srep[p][:, j * T + cols.start:j * T + cols.stop]

                def sim_v(g, cols):
                    p, j = divmod(g, 2)
                    return simp[p][:, j * T + cols.start:j * T + cols.stop]
            else:
                sre = [states.tile([128, T], sdt, tag=f"sre{g}",
                                   name=f"sre{g}") for g in range(NG)]
                sim = [states.tile([128, T], sdt, tag=f"sim{g}",
                                   name=f"sim{g}") for g in range(NG)]

                def sre_v(g, cols):
                    return sre[g][:, cols]

                def sim_v(g, cols):
                    return sim[g][:, cols]
            gdt = sdt if TAIL_V2 else f32
            gsigs = [states.tile([128, OUT], gdt, tag=f"gsig{ti}",
                                 name=f"gsig{ti}") for ti in range(NT)]
            skips = [states.tile([128, OUT], gdt, tag=f"skip{ti}",
                                 name=f"skip{ti}") for ti in range(NT)]

            # ---- stage A: gated = (pre + bpre) * sig(gin + bgin) ----
            # [W_pre | W_gin] packed on partitions: one matmul fills both
            gated = singles.tile([TR, T], sdt, tag="gated")
            with (
                tc.tile_pool(name="psumA", bufs=1, space="PSUM") as psumA,
                tc.tile_pool(name="wkA", bufs=2) as wkA,
            ):
                pg_ps = psumA.tile([2 * TR, T], f32, tag="pg")
                for h in range(2):
                    cols = slice(h * 512, (h + 1) * 512)
                    for k in range(KD):
                        nc.tensor.matmul(pg_ps[:, cols], Wpg_sb[k],
                                         xT_sb[k][:, cols],
                                         start=(k == 0), stop=(k == KD - 1))
                gsigA = wkA.tile([TR, T], f32, tag="gsigA")
                for h in range(2):
                    cols = slice(h * 512, (h + 1) * 512)
                    nc.scalar.activation(gsigA[:, cols],
                                         pg_ps[TR:2 * TR, cols],
                                         AF.Sigmoid, bias=bgin_sb, scale=1.0)
                nc.vector.scalar_tensor_tensor(
                    out=gated, in0=pg_ps[0:TR, :], scalar=bpre_sb, in1=gsigA,
                    op0=op.add, op1=op.mult)

            NTE = globals().get("NTE_OVERRIDE", 7)  # stage-B psum tiles
            # ---- stage B: scans per channel group + gout/skip fill ----
            if STAGES == "A":
                fin = states.tile([128, OUT], f32, tag="fin", name="fin")
                nc.vector.memset(fin, 0.5)
                nc.vector.scalar_tensor_tensor(
                    out=fin, in0=pre_ps if False else fin, scalar=bpre_sb[0:1, 0:1] if False else 1.0,
                    in1=fin, op0=op.mult, op1=op.mult)
                nc.sync.dma_start(out=out_d[0:128, :], in_=fin)
            if STAGES != "A":
              with (
                tc.tile_pool(name="psumG", bufs=1, space="PSUM") as psumG,
                tc.tile_pool(name="psumM", bufs=1, space="PSUM") as psumM,
                tc.tile_pool(name="wkC", bufs=3) as wkC,
                tc.tile_pool(name="wkB", bufs=3) as wkB,
            ):
                zms = [psumM.tile([128, OUT], f32, tag=f"zm{ti}",
                                  name=f"zm{ti}") for ti in range(NTE)]

                def ln_tail(ti, zm, wk=None):
                    wk = wk if wk is not None else wkC
                    gsig = gsigs[ti]
                    if TAIL_V2:
                        # h via STT with free-dim sum; var from ACT
                        # Square+accum; bf16 tail ops (DVE 2x)
                        h_t = wk.tile([128, OUT], sdt, tag="h", name="h_t")
                        sumh = wk.tile([128, 1], f32, tag="sh",
                                       name="sumh")
                        nc.vector.scalar_tensor_tensor(
                            out=h_t, in0=zm, scalar=1.0, in1=gsig,
                            op0=op.mult, op1=op.mult, accum_out=sumh)
                        sq = wk.tile([128, OUT], sdt, tag="sqs", name="sq")
                        sumh2 = wk.tile([128, 1], f32, tag="sh2",
                                        name="sumh2")
                        nc.scalar.activation(sq, h_t, AF.Square,
                                             accum_out=sumh2)
                        mu = wk.tile([128, 1], f32, tag="mu", name="mu")
                        nc.vector.tensor_scalar(mu, sumh, 1.0 / OUT, None,
                                                op.mult)
                        q0 = wk.tile([128, 1], f32, tag="q0", name="q0")
                        nc.vector.tensor_tensor(q0, mu, sumh, op.mult)
                        q1 = wk.tile([128, 1], f32, tag="q1", name="q1")
                        nc.vector.tensor_tensor(q1, q0, sumh2, op.subtract)
                        sd = wk.tile([128, 1], f32, tag="sd", name="sd")
                        nc.scalar.activation(sd, q1, AF.Sqrt, bias=eps_sb,
                                             scale=-1.0 / OUT)
                        rstd = wk.tile([128, 1], f32, tag="rstd",
                                       name="rstd")
                        nc.vector.reciprocal(rstd, sd)
                        beta = wk.tile([128, 1], f32, tag="beta",
                                       name="beta")
                        nc.vector.scalar_tensor_tensor(
                            out=beta, in0=mu, scalar=-1.0, in1=rstd,
                            op0=op.mult, op1=op.mult)
                        ln = wk.tile([128, OUT], sdt, tag="ln", name="ln")
                        nc.scalar.activation(ln, h_t, AF.Identity,
                                             bias=beta, scale=rstd)
                        omg = wk.tile([128, OUT], sdt, tag="omg",
                                      name="omg")
                        nc.scalar.activation(omg, gsig, AF.Copy,
                                             bias=1.0, scale=-1.0)
                        sk2 = wk.tile([128, OUT], sdt, tag="sk2",
                                      name="sk2")
                        nc.vector.tensor_tensor(sk2, omg, skips[ti],
                                                op.mult)
                        outt = wk.tile([128, OUT], f32, tag="outt",
                                       name="outt")
                        nc.vector.tensor_tensor(outt, ln, sk2, op.add)
                        nc.sync.dma_start(
                            out=out_d[ti * 128:(ti + 1) * 128, :],
                            in_=outt)
                        return
                    h_t = wk.tile([128, OUT], f32, tag="h", name="h_t")
                    nc.vector.tensor_tensor(h_t, gsig, zm, op.mult)
                    stats = wk.tile([128, 6], f32, tag="stats", name="stats")
                    nc.vector.bn_stats(stats, h_t)
                    mv = wk.tile([128, 2], f32, tag="mv", name="mv")
                    nc.vector.bn_aggr(mv, stats)
                    sd = wk.tile([128, 1], f32, tag="sd", name="sd")
                    nc.scalar.activation(sd, mv[:, 1:2], AF.Sqrt,
                                         bias=eps_sb, scale=1.0)
                    rstd = wk.tile([128, 1], f32, tag="rstd", name="rstd")
                    nc.vector.reciprocal(rstd, sd)
                    beta = wk.tile([128, 1], f32, tag="beta", name="beta")
                    nc.vector.scalar_tensor_tensor(
                        out=beta, in0=mv[:, 0:1], scalar=-1.0, in1=rstd,
                        op0=op.mult, op1=op.mult)
                    ln = wk.tile([128, OUT], f32, tag="ln", name="ln")
                    nc.scalar.activation(ln, h_t, AF.Identity,
                                         bias=beta, scale=rstd)
                    omg = wk.tile([128, OUT], f32, tag="omg", name="omg")
                    nc.scalar.activation(omg, gsig, AF.Copy,
                                         bias=1.0, scale=-1.0)
                    sk2 = wk.tile([128, OUT], f32, tag="sk2", name="sk2")
                    nc.vector.tensor_tensor(sk2, omg, skips[ti], op.mult)
                    outt = wk.tile([128, OUT], f32, tag="outt", name="outt")
                    nc.vector.tensor_tensor(outt, ln, sk2, op.add)
                    nc.sync.dma_start(out=out_d[ti * 128:(ti + 1) * 128, :],
                                      in_=outt)
                for g in range(NG):
                    # broadcast gated rows m -> 16 c-rows via DMA (no PE)
                    zxs = wkB.tile([128, T], sdt, tag="zxs")
                    if zx_dma:
                        # SWDGE (gpsimd) queue: the HWDGE path splits one
                        # DMA across 16 SDMA engines and its completion
                        # semaphore can fire before all replicated writes
                        # land; the software-descgen path is safe.
                        nc.gpsimd.dma_start(
                            out=zxs,
                            in_=gated[8 * g:8 * g + 8, :].unsqueeze(1)
                            .broadcast_to((8, 16, T)))
                    else:
                        with tc.tile_pool(name="psumB", bufs=2,
                                          space="PSUM") as psumB:
                            for h in range(2):
                                cols = slice(h * 512, (h + 1) * 512)
                                zx = psumB.tile([128, 512], f32, tag="zx",
                                                name="zx")
                                nc.tensor.matmul(
                                    zx,
                                    EXPM_sb[:, g * 128:(g + 1) * 128],
                                    gated[:, cols], start=True, stop=True)
                                nc.scalar.activation(zxs[:, cols], zx,
                                                     AF.Copy)
                    inA = wkB.tile([128, T], sdt, tag="mod")
                    inB = wkB.tile([128, T], sdt, tag="mod")
                    nc.vector.tensor_tensor(inA, COS_sb, zxs, op.mult)
                    nc.vector.tensor_tensor(inB, SIN_sb, zxs, op.mult)
                    if MOD_FUSE:
                        pj, jj = divmod(g, 2)
                        if jj == 0:
                            ap_t = wkB.tile([128, 2 * T], sdt, tag="scn2")
                            bp_t = wkB.tile([128, 2 * T], sdt, tag="scn2")
                        a_t = ap_t[:, jj * T:(jj + 1) * T]
                        b_t = bp_t[:, jj * T:(jj + 1) * T]
                    else:
                        a_t = wkB.tile([128, T], sdt, tag="scn")
                        b_t = wkB.tile([128, T], sdt, tag="scn")
                    dec_b = DECF_sb[:, g * T:(g + 1) * T]
                    nc.vector.tensor_tensor_scan(
                        a_t, dec_b, inA, 0.0, op.mult, op.add)
                    (nc.gpsimd if POOL_SCAN else nc.vector).tensor_tensor_scan(
                        b_t, dec_b, inB, 0.0, op.mult, op.add)
                    if MOD_FUSE and jj == 0:
                        pass  # post-mods+mix deferred to the odd group
                    elif MOD_FUSE:
                        # fused pair post-mods on [128, 2T]
                        p1 = wkB.tile([128, 2 * T], sdt, tag="mod2")
                        p2 = wkB.tile([128, 2 * T], sdt, tag="mod2")
                        p3 = wkB.tile([128, 2 * T], sdt, tag="mod2")
                        p4 = wkB.tile([128, 2 * T], sdt, tag="mod2")
                        nc.vector.tensor_tensor(p1, COS2_sb, ap_t, op.mult)
                        nc.vector.tensor_tensor(p2, SIN2_sb, bp_t, op.mult)
                        nc.vector.tensor_tensor(srep[pj], p1, p2, op.add)
                        nc.vector.tensor_tensor(p3, SIN2_sb, ap_t, op.mult)
                        nc.vector.tensor_tensor(p4, COS2_sb, bp_t, op.mult)
                        nc.vector.tensor_tensor(simp[pj], p3, p4,
                                                op.subtract)
                    else:
                        # state_re = COS*A + SIN*B   (DVE)
                        p1 = wkB.tile([128, T], sdt, tag="mod")
                        p2 = wkB.tile([128, T], sdt, tag="mod")
                        nc.vector.tensor_tensor(p1, COS_sb, a_t, op.mult)
                        (nc.gpsimd if POOL_P2 else nc.vector).tensor_tensor(
                            p2, SIN_sb, b_t, op.mult)
                        nc.vector.tensor_tensor(sre[g], p1, p2, op.add)
                        # state_im = SIN*A - COS*B
                        PIM = nc.gpsimd if POOL_IM else nc.vector
                        p3 = wkB.tile([128, T], sdt, tag="pim")
                        p4 = wkB.tile([128, T], sdt, tag="pim")
                        nc.vector.tensor_tensor(p3, SIN_sb, a_t, op.mult)
                        PIM.tensor_tensor(p4, COS_sb, b_t, op.mult)
                        PIM.tensor_tensor(sim[g], p3, p4, op.subtract)
                    # gout/skip matmuls for token tile g fill PE idle time
                    ti = g
                    tcols = slice(ti * 128, (ti + 1) * 128)
                    gout_ps = psumG.tile([128, OUT], f32, tag="gout",
                                         name="gout_ps")
                    if gs_fp8:
                        for k in range(0, KD, 2):
                            nc.tensor.matmul(gout_ps,
                                             xT8_sb[:, k:k + 2, tcols],
                                             Wg8_sb[:, k:k + 2, :],
                                             start=(k == 0),
                                             stop=(not wb and k == KD - 2),
                                             perf_mode=PM.DoubleRow)
                    else:
                        for k in range(KD):
                            nc.tensor.matmul(gout_ps, xT_sb[k][:, tcols],
                                             Wgout_sb[k], start=(k == 0),
                                             stop=(not wb and k == KD - 1))
                    if wb:
                        nc.tensor.matmul(gout_ps, r(ones_sb), r(bgout_sb),
                                         start=False, stop=True)
                    nc.scalar.activation(gsigs[ti], gout_ps, AF.Sigmoid,
                                         scale=DSC if gs_fp8 else 1.0)
                    skip_ps = psumG.tile([128, OUT], f32, tag="gout",
                                         name="skip_ps")
                    if gs_fp8:
                        for k in range(0, KD, 2):
                            nc.tensor.matmul(skip_ps,
                                             xT8_sb[:, k:k + 2, tcols],
                                             Ws8_sb[:, k:k + 2, :],
                                             start=(k == 0),
                                             stop=(not wb and k == KD - 2),
                                             perf_mode=PM.DoubleRow)
                    else:
                        for k in range(KD):
                            nc.tensor.matmul(skip_ps, xT_sb[k][:, tcols],
                                             Wskip_sb[k], start=(k == 0),
                                             stop=(not wb and k == KD - 1))
                    if wb:
                        nc.tensor.matmul(skip_ps, r(ones_sb), r(bskip_sb),
                                         start=False, stop=True)
                    nc.scalar.activation(skips[ti], skip_ps, AF.Copy,
                                         scale=DSC if gs_fp8 else 1.0)
                    mm_groups = () if (MOD_FUSE and g % 2 == 0) else (
                        (g - 1, g) if MOD_FUSE else (g,))
                    for gg in mm_groups:
                        for tj in range(NTE):
                            tc2 = slice(tj * 128, (tj + 1) * 128)
                            nc.tensor.matmul(zms[tj], sre_v(gg, tc2),
                                             Wmre_sb[gg], start=(gg == 0),
                                             stop=False,
                                             skip_group_check=True)
                            nc.tensor.matmul(zms[tj], sim_v(gg, tc2),
                                             Wmim_sb[gg], start=False,
                                             stop=(not wb and gg == NG - 1),
                                             skip_group_check=True)
                    if g == NG - 1:
                        for tj in range(NTE):
                            if wb:
                                nc.tensor.matmul(zms[tj], r(ones_sb),
                                                 r(bmix_sb), start=False,
                                                 stop=True,
                                                 skip_group_check=True)
                            ln_tail(tj, zms[tj])

              if STAGES == "B":
                fin = states.tile([128, OUT], f32, tag="fin", name="fin")
                nc.vector.tensor_tensor(fin, skips[0], gsigs[0], op.mult)
                for g in range(NG):
                    nc.vector.tensor_tensor(fin, sre[g][:, 0:OUT],
                                            sim[g][:, 0:OUT], op.mult)
                nc.sync.dma_start(out=out_d[0:128, :], in_=fin)
            # ---- stage C: remaining mix tiles + LN tail ----
            if STAGES == "ALL":
              with (
                tc.tile_pool(name="psumC", bufs=3, space="PSUM") as psumC,
                tc.tile_pool(name="wkC2", bufs=3) as wkC2,
            ):
                for ti in range(NTE, NT):
                    tcols = slice(ti * 128, (ti + 1) * 128)
                    zm = psumC.tile([128, OUT], f32, tag="zm", name="zm")
                    for g in range(NG):
                        nc.tensor.matmul(zm, sre_v(g, tcols), Wmre_sb[g],
                                         start=(g == 0), stop=False)
                        nc.tensor.matmul(zm, sim_v(g, tcols), Wmim_sb[g],
                                         start=False,
                                         stop=(not wb and g == NG - 1))
                    if wb:
                        nc.tensor.matmul(zm, r(ones_sb), r(bmix_sb),
                                         start=False, stop=True)
                    ln_tail(ti, zm, wkC2)

    nc.compile()
    return nc


def host_prep(inputs):
    """Compute per-core input maps from the full problem inputs."""
    import ml_dtypes

    sdt_np = ml_dtypes.bfloat16 if STREAM_BF16 else np.float32

    x = np.asarray(inputs["x"], np.float32)
    a = np.abs(np.asarray(inputs["ffa_a"], np.float64))       # [TR]
    b = np.asarray(inputs["ffa_b"], np.float64)               # [CTX]
    t = np.arange(T, dtype=np.float64)

    cos_cols = np.cos(b[:, None] * t[None, :])                # [CTX, T]
    sin_cols = np.sin(b[:, None] * t[None, :])
    COS = np.tile(cos_cols, (8, 1)).astype(sdt_np)            # [128, T]
    SIN = np.tile(sin_cols, (8, 1)).astype(sdt_np)

    dec = np.exp(-a).astype(np.float32)                       # [TR]
    rr = np.arange(128)
    DEC = np.empty((128, NG), np.float32)
    for g in range(NG):
        DEC[:, g] = dec[8 * g + rr // 16]
    DECF = np.empty((128, NG * T), np.float32)
    for g in range(NG):
        DECF[:, g * T:(g + 1) * T] = DEC[:, g][:, None]

    Wm = np.asarray(inputs["W_mix"], np.float32).reshape(TR, 2, CTX, OUT)
    Wmre = np.ascontiguousarray(Wm[:, 0].reshape(NCH, OUT)).astype(sdt_np)
    Wmim = np.ascontiguousarray(Wm[:, 1].reshape(NCH, OUT)).astype(sdt_np)

    Wpg = np.concatenate(
        [np.asarray(inputs["W_pre"], np.float32),
         np.asarray(inputs["W_gin"], np.float32)], axis=1)

    col = np.arange(NCH)
    EXPM = (np.arange(TR)[:, None] == (col[None, :] // CTX)).astype(sdt_np)

    # fp8 DoubleRow operands: scale by powers of 2, descale 2^-16 on-chip
    SX, SW = 32.0, 2048.0
    f8 = ml_dtypes.float8_e4m3fn

    def dr_pack(a, scale):  # [D, N] -> [Ki=128, Ko=KD, N] with d = Ko*128+Ki
        q = (np.asarray(a, np.float32) * scale).astype(f8)
        return np.ascontiguousarray(
            q.reshape(KD, 128, a.shape[1]).transpose(1, 0, 2))

    Wg8 = dr_pack(inputs["W_gout"], SW)
    Ws8 = dr_pack(inputs["W_skip"], SW)

    shared = {
        "Wpg": np.ascontiguousarray(Wpg).astype(sdt_np),
        "EXPM": EXPM,
        "Wg8": Wg8, "Ws8": Ws8,
        "Wgout": np.ascontiguousarray(inputs["W_gout"], np.float32).astype(sdt_np),
        "Wskip": np.ascontiguousarray(inputs["W_skip"], np.float32).astype(sdt_np),
        "Wmre": Wmre, "Wmim": Wmim,
        "COS": COS, "SIN": SIN, "DEC": DEC,
        "DECF": DECF.astype(sdt_np),
        "bpre": np.asarray(inputs["b_pre"], np.float32).reshape(TR, 1),
        "bgin": np.asarray(inputs["b_gin"], np.float32).reshape(TR, 1),
        # fp8 path: PSUM holds (SX*SW)*x@W; Act applies 2^-16, so biases
        # folded in via the ones-matmul must be pre-scaled by 2^16.
        "bgout": np.asarray(inputs["b_gout"], np.float32).reshape(1, OUT)
        * (SX * SW),
        "bskip": np.asarray(inputs["b_skip"], np.float32).reshape(1, OUT)
        * (SX * SW),
        "bmix": np.asarray(inputs["b_mix"], np.float32).reshape(1, OUT),
        "ones": np.ones((1, 128), np.float32),
    }
    in_maps = []
    for core in range(B):
        m = dict(shared)
        xTc = np.ascontiguousarray(x[core].T)
        m["xT"] = xTc.astype(sdt_np)
        m["xT8"] = dr_pack(xTc, SX)
        in_maps.append(m)
    return in_maps


def kernel(**inputs):
    from concourse import bass_utils

    wb = any(
        np.any(np.asarray(inputs[k]))
        for k in ("b_pre", "b_gin", "b_gout", "b_skip", "b_mix")
    )
    key = f"nc_wb{wb}"
    if key not in _cache:
        _cache[key] = build_program(with_bias=wb)
    nc = _cache[key]
    in_maps = host_prep(inputs)
    res = bass_utils.run_bass_kernel_spmd(nc, in_maps, core_ids=list(range(B)))
    return np.stack([res.results[i]["out"] for i in range(B)])

